# revision 4
# baseline (speedup 1.0000x reference)
"""LIF spike (leaky integrate-and-fire) forward kernel for Trainium2.

Recurrence over the innermost time axis T=8 of x[64,128,32,32,8] (fp32):
    u_t = TAU * u_{t-1} * (1 - o_{t-1}) + x_t
    o_t = (u_t > VTH)
Data-parallel over the batch dim: 8 NeuronCores x 8 batches each.

Per-core layout: the 32 MiB shard is viewed as [2048 rows, 4096 cols]
(each row = one (b, c, h-half) slab, cols = spatial*T contiguous). Tiles
of [128, 4096] stream HBM->SBUF; inside SBUF the recurrence walks the
stride-8 time slices in place (the x tile doubles as the membrane-state
buffer), spikes are produced by ScalarE (Sign+Relu) while VectorE does the
reset (copy_predicated) and leak+integrate (scalar_tensor_tensor), so the
kernel stays DMA-bound.
"""

import sys

for _p in ("/opt/trn_rl_repo",):
    if _p not in sys.path:
        sys.path.insert(0, _p)

import numpy as np

TAU = 0.1
VTH = 1.5

B, C, H, W, T = 64, 128, 32, 32, 8
NCORES = 8
BS = B // NCORES                      # batches per core
ELEMS = BS * C * H * W * T            # 8,388,608 per core
FREE = 4096                           # tile free dim (cols)
ROWS = ELEMS // FREE                  # 2048
S = FREE // T                         # 512 spatial elems per time slice
P = 128                               # partitions
NTILES = ROWS // P                    # 16

_compiled = None


def _build():
    import concourse.bacc as bacc
    import concourse.mybir as mybir
    import concourse.tile as tile

    nc = bacc.Bacc(
        "TRN2",
        target_bir_lowering=False,
        debug=False,
        num_devices=NCORES,
    )
    f32 = mybir.dt.float32
    x_d = nc.dram_tensor("x", [ROWS, FREE], f32, kind="ExternalInput").ap()
    o_d = nc.dram_tensor("o", [ROWS, FREE], f32, kind="ExternalOutput").ap()

    i8 = mybir.dt.int8
    mult = mybir.AluOpType.mult
    add = mybir.AluOpType.add
    is_gt = mybir.AluOpType.is_gt

    with tile.TileContext(nc) as tc:
        with (
            tc.tile_pool(name="xp", bufs=3) as xp,
            tc.tile_pool(name="mp", bufs=3) as mp,
            tc.tile_pool(name="op", bufs=3) as op_,
            tc.tile_pool(name="zc", bufs=1) as zp,
        ):
            zero = zp.tile([P, S], f32)
            nc.gpsimd.memset(zero[:], 0.0)
            for i in range(NTILES):
                xt = xp.tile([P, FREE], f32)
                nc.sync.dma_start(out=xt[:], in_=x_d[i * P : (i + 1) * P, :])
                mt = mp.tile([P, FREE], i8)
                xv = xt[:].rearrange("p (s t) -> p t s", t=T)
                mv = mt[:].rearrange("p (s t) -> p t s", t=T)
                for t in range(T):
                    u = xv[:, t]
                    if t > 0:
                        up = xv[:, t - 1]
                        # reset where previous step spiked
                        nc.vector.copy_predicated(
                            out=up, mask=mv[:, t - 1], data=zero[:]
                        )
                        # u_t = TAU * u_{t-1} + x_t   (in place into x slice t)
                        nc.vector.scalar_tensor_tensor(
                            out=u, in0=up, scalar=TAU, in1=u, op0=mult, op1=add
                        )
                    # o_t = (u_t > VTH) as int8 {0,1}
                    nc.vector.tensor_scalar(mv[:, t], u, VTH, None, is_gt)
                # int8 {0,1} -> fp32 spikes, whole tile in one ACT copy
                ot = op_.tile([P, FREE], f32)
                nc.scalar.copy(ot[:], mt[:])
                nc.sync.dma_start(out=o_d[i * P : (i + 1) * P, :], in_=ot[:])
    nc.compile()
    return nc


def _get_compiled():
    global _compiled
    if _compiled is None:
        _compiled = _build()
    return _compiled


def kernel(x: np.ndarray, _trace: bool = False):
    nc = _get_compiled()
    from concourse.bass_utils import run_bass_kernel_spmd

    x = np.asarray(x, dtype=np.float32)
    in_maps = [
        {"x": np.ascontiguousarray(x[i * BS : (i + 1) * BS]).reshape(ROWS, FREE)}
        for i in range(NCORES)
    ]
    res = run_bass_kernel_spmd(
        nc, in_maps, core_ids=list(range(NCORES)), trace=_trace
    )
    out = np.concatenate(
        [r["o"].reshape(BS, C, H, W, T) for r in res.results], axis=0
    )
    if _trace:
        return out, res
    return out


# revision 7
# speedup vs baseline: 2.3353x; 2.3353x over previous
"""LIF spike (leaky integrate-and-fire) forward kernel for Trainium2.

Recurrence over the innermost time axis T=8 of x[64,128,32,32,8] (fp32):
    u_t = TAU * u_{t-1} * (1 - o_{t-1}) + x_t
    o_t = (u_t > VTH)
Data-parallel over the batch dim: 8 NeuronCores x 8 batches each.

Per-core layout: the 32 MiB shard is viewed as [2048 rows, 4096 cols]
(each row = one (b, c, h-half) slab, cols = spatial*T contiguous). Tiles
of [128, 4096] stream HBM->SBUF; inside SBUF the recurrence walks the
stride-8 time slices in place (the x tile doubles as the membrane-state
buffer), spikes are produced by ScalarE (Sign+Relu) while VectorE does the
reset (copy_predicated) and leak+integrate (scalar_tensor_tensor), so the
kernel stays DMA-bound.
"""

import sys

for _p in ("/opt/trn_rl_repo",):
    if _p not in sys.path:
        sys.path.insert(0, _p)

import numpy as np

TAU = 0.1
VTH = 1.5

B, C, H, W, T = 64, 128, 32, 32, 8
NCORES = 8
BS = B // NCORES                      # batches per core
ELEMS = BS * C * H * W * T            # 8,388,608 per core
FREE = 4096                           # tile free dim (cols)
ROWS = ELEMS // FREE                  # 2048
S = FREE // T                         # 512 spatial elems per time slice
P = 128                               # partitions
NTILES = ROWS // P                    # 16

_compiled = None


def _build(reps: int = 1):
    import contextlib

    import concourse.bacc as bacc
    import concourse.mybir as mybir
    import concourse.tile as tile

    nc = bacc.Bacc(
        "TRN2",
        target_bir_lowering=False,
        debug=False,
        num_devices=NCORES,
    )
    f32 = mybir.dt.float32
    x_d = nc.dram_tensor("x", [ROWS, FREE], f32, kind="ExternalInput").ap()
    o_d = nc.dram_tensor("o", [ROWS, FREE], f32, kind="ExternalOutput").ap()

    i8 = mybir.dt.int8
    mult = mybir.AluOpType.mult
    add = mybir.AluOpType.add
    is_gt = mybir.AluOpType.is_gt

    with tile.TileContext(nc) as tc:
        with (
            tc.tile_pool(name="xp", bufs=3) as xp,
            tc.tile_pool(name="mp", bufs=3) as mp,
            tc.tile_pool(name="op", bufs=3) as op_,
            tc.tile_pool(name="zc", bufs=1) as zp,
        ):
            zero = zp.tile([P, S], f32)
            nc.gpsimd.memset(zero[:], 0.0)
            rep_ctx = (
                tc.For_i(0, reps, 1) if reps > 1 else contextlib.nullcontext()
            )
            with rep_ctx:
                _emit_tiles(nc, tc, xp, mp, op_, zero, x_d, o_d, mybir)
    nc.compile()
    return nc


def _emit_tiles(nc, tc, xp, mp, op_, zero, x_d, o_d, mybir):
    f32 = mybir.dt.float32
    i8 = mybir.dt.int8
    mult = mybir.AluOpType.mult
    add = mybir.AluOpType.add
    is_gt = mybir.AluOpType.is_gt
    if True:
            for i in range(NTILES):
                xt = xp.tile([P, FREE], f32)
                nc.sync.dma_start(out=xt[:], in_=x_d[i * P : (i + 1) * P, :])
                mt = mp.tile([P, FREE], i8)
                xv = xt[:].rearrange("p (s t) -> p t s", t=T)
                mv = mt[:].rearrange("p (s t) -> p t s", t=T)
                for t in range(T):
                    u = xv[:, t]
                    if t > 0:
                        up = xv[:, t - 1]
                        # reset where previous step spiked
                        nc.vector.copy_predicated(
                            out=up, mask=mv[:, t - 1], data=zero[:]
                        )
                        # u_t = TAU * u_{t-1} + x_t   (in place into x slice t)
                        nc.vector.scalar_tensor_tensor(
                            out=u, in0=up, scalar=TAU, in1=u, op0=mult, op1=add
                        )
                    # o_t = (u_t > VTH) as int8 {0,1}
                    nc.vector.tensor_scalar(mv[:, t], u, VTH, None, is_gt)
                # int8 {0,1} -> fp32 spikes, whole tile in one ACT copy
                ot = op_.tile([P, FREE], f32)
                nc.scalar.copy(ot[:], mt[:])
                nc.sync.dma_start(out=o_d[i * P : (i + 1) * P, :], in_=ot[:])


def _get_compiled():
    global _compiled
    if _compiled is None:
        _compiled = _build()
    return _compiled


def kernel(x: np.ndarray, _trace: bool = False):
    nc = _get_compiled()
    from concourse.bass_utils import run_bass_kernel_spmd

    x = np.asarray(x, dtype=np.float32)
    in_maps = [
        {"x": np.ascontiguousarray(x[i * BS : (i + 1) * BS]).reshape(ROWS, FREE)}
        for i in range(NCORES)
    ]
    res = run_bass_kernel_spmd(
        nc, in_maps, core_ids=list(range(NCORES)), trace=_trace
    )
    out = np.concatenate(
        [r["o"].reshape(BS, C, H, W, T) for r in res.results], axis=0
    )
    if _trace:
        return out, res
    return out
